# revision 32
# baseline (speedup 1.0000x reference)
"""DilatedAttention Trainium2 Bass kernel.

Reference math (B=2, S=2048, D=512, rates (1,2,4,8)):
  Q=xWq+bq K=xWk+bk V=xWv+bv
  scores = QK^T/sqrt(D); per rate r: mask j%r==0 & j<=i, softmax, ctx=attn@V,
  out_r = ctx@Wr[r]+br[r]; output = concat(out_r)@Wo + bo
  avg_attention = mean_r attn_r

Kernel strategy (8 NeuronCores, one uniform SPMD program):
  - core c handles batch c//4 with TWO 256-row query blocks m'=c%4 and 7-m'
    (the pairing balances causal work across cores; block0 rows < 1024).
  - host folds weights:  W2 = Wk@Wq^T  (scoresT[j,i] = x_j W2 x_i^T)
                         Wfold_r = Wv@Wr[r]@Wo_r
                         bias_out = bo + sum_r (bv@Wr[r]+br[r])@Wo_r
    (softmax rows summing to 1 turns the bv term into a constant; bq=bk=0
    asserted - they are structurally zero in this model.)
  - device: GT = W2^T@xT; transposed-layout scores per dilation *group*
    (A: j%8==0, B: j%8==4, C: j%4==2, D: j%2==1 -> strided SBUF column reads;
    a group is a residue class, so per-rate row sets are unions of leading
    groups), causal mask via scalar_tensor_tensor index compare, group
    denominators via one small matmul per slot, avg_attention produced by
    TensorE-transposing the masked exp tiles and scaling with the
    per-partition reciprocal-denominator combination of the slot's class,
    Y_r = expT_r @ (x_r@Wfold_r) with 1/denom fused into the output
    accumulation (scalar_tensor_tensor).
  - all matmuls float32r.
"""
import sys

sys.path.insert(0, "/opt/trn_rl_repo")

import math
import numpy as np

import concourse.bacc as bacc
import concourse.mybir as mybir
import concourse.tile as tile
from concourse.bass_utils import run_bass_kernel_spmd
from concourse.masks import make_identity

B, S, D = 2, 2048, 512
N_CORES = 8
ROWS = 512  # query rows per core (2 blocks of 256)
BW = 256  # block width
SQRT_D = math.sqrt(D)
F32 = mybir.dt.float32
F32R = mybir.dt.float32r

# dilation groups: j = stride*(128*local + p) + off
G_STRIDE = [8, 8, 4, 2]
G_OFF = [0, 4, 2, 1]
# block0 slot list sized for J<=1024; block1 full; block0 subset of block1
SLOTS_B0 = [(0, 0), (1, 0), (2, 0), (2, 1), (3, 0), (3, 1), (3, 2), (3, 3)]
SLOTS_B1 = (
    [(0, l) for l in range(2)] + [(1, l) for l in range(2)]
    + [(2, l) for l in range(4)] + [(3, l) for l in range(8)]
)
BLK_SLOTS = [SLOTS_B0, SLOTS_B1]
SHARED = set(SLOTS_B0)
N_SLOTS = len(SLOTS_B1)  # 16 physical slots; shared ones hold both blocks
VP_COUNTS = [2, 2, 4, 8]
VP_BASE = [0, 16, 24, 28]
N_VP = 30
N_IT = 4  # i-tiles of 128 (2 per block)
RATE_GROUPS = [4, 3, 2, 1]  # ridx (rates 1,2,4,8) -> number of leading groups


def vp_index(ridx, g, l):
    return VP_BASE[ridx] + sum(VP_COUNTS[:g]) + l


def slot_j(g, l, p):
    return G_STRIDE[g] * (128 * l + p) + G_OFF[g]


def build_program():
    nc = bacc.Bacc(
        "TRN2", target_bir_lowering=False, debug=False, num_devices=N_CORES
    )
    d = {}

    def inp(name, shape, dt=F32):
        d[name] = nc.dram_tensor(name, shape, dt, kind="ExternalInput").ap()

    inp("xtm", [D, S], F32R)
    inp("xtq", [D, ROWS], F32R)
    inp("w2", [D, D], F32R)
    inp("wfold", [4 * D, D], F32R)
    inp("iidx", [128, ROWS])
    inp("jidx", [128, N_SLOTS])
    inp("slotsel", [128, N_SLOTS, 4], F32R)
    inp("bias_tile", [128, D])
    inp("identr", [128, 128], F32R)
    out_d = nc.dram_tensor("out", [ROWS, D], F32, kind="ExternalOutput").ap()
    avg_d = nc.dram_tensor("avg", [ROWS, S], F32, kind="ExternalOutput").ap()

    with tile.TileContext(nc) as tc:
        build_core(tc, d, out_d, avg_d)
    nc.compile()
    return nc


def build_core(tc, d, out_d, avg_d):
    nc = tc.nc
    Exp = mybir.ActivationFunctionType.Exp

    with (
        tc.tile_pool(name="consts", bufs=1) as cpool,
        tc.tile_pool(name="gt", bufs=1) as gt_pool,
        tc.tile_pool(name="vp", bufs=1) as vp_pool,
        tc.tile_pool(name="psmm", bufs=3, space="PSUM") as pp,
        tc.tile_pool(name="psacc", bufs=1, space="PSUM") as pp_acc,
    ):
        gt_sb = gt_pool.tile([128, 4, S], F32R)
        vp_sb = vp_pool.tile([128, N_VP, D], F32R)

        # ------------- phases 1+2: GT and V' (xtm/weights scoped) -------------
        with (
            tc.tile_pool(name="xtm", bufs=1) as xtm_pool,
            tc.tile_pool(name="wstream", bufs=2) as wstream,
        ):
            w2_sb = wstream.tile([128, 4, D], F32R, tag="w")
            w2_dram = d["w2"].rearrange("(dt p) c -> p dt c", p=128)
            xtm_sb = xtm_pool.tile([128, 4, S], F32R)
            xtm_dram = d["xtm"].rearrange("(dt p) c -> p dt c", p=128)
            xtm0_sb = wstream.tile([128, 4, 512], F32R, tag="x0")
            xtm1_sb = wstream.tile([128, 4, 512], F32R, tag="x1")
            for dt in range(4):
                nc.sync.dma_start(w2_sb[:, dt, :], w2_dram[:, dt, :])
                nc.sync.dma_start(xtm0_sb[:, dt, :], xtm_dram[:, dt, 0:512])
            nc.sync.dma_start(xtm1_sb[:], xtm_dram[:, :, 512:1024])
            xtq_sb = cpool.tile([128, 4, ROWS], F32R, tag="xtq")
            nc.scalar.dma_start(
                xtq_sb[:], d["xtq"].rearrange("(dt p) c -> p dt c", p=128)
            )
            for jc4 in range(4):
                nc.sync.dma_start(
                    xtm_sb[:, :, jc4 * 512 : jc4 * 512 + 512],
                    xtm_dram[:, :, jc4 * 512 : jc4 * 512 + 512],
                )

            # GT[dout, j] = W2^T @ xT  (dt outermost so the first matmuls only
            # need the dt=0 slices; chunk 0 reads its own small tile)
            for jc in range(4):
                col = jc * 512
                psa = pp.tile([128, 1024], F32, tag="mm")
                psb = pp.tile([128, 1024], F32, tag="mm")
                for dt in range(4):
                    for dto in range(4):
                        ps = psa if dto < 2 else psb
                        h = (dto % 2) * 512
                        nc.tensor.matmul(
                            ps[:, h : h + 512],
                            w2_sb[:, dt, dto * 128 : dto * 128 + 128],
                            (
                                [xtm0_sb, xtm1_sb][jc][:, dt, :]
                                if jc < 2
                                else xtm_sb[:, dt, col : col + 512]
                            ),
                            start=(dt == 0),
                            stop=(dt == 3),
                        )
                for dto in range(4):
                    ps = psa if dto < 2 else psb
                    h = (dto % 2) * 512
                    dst = gt_sb[:, dto, col : col + 512]
                    if dto % 2 == 0:
                        nc.scalar.copy(dst, ps[:, h : h + 512])
                    else:
                        nc.vector.tensor_copy(dst, ps[:, h : h + 512])

            # V'[vp slot] = x_rows(group tile) @ Wfold_r
            for ridx in range(4):
                wf_sb = wstream.tile([128, 4, D], F32R, tag="w")
                nc.scalar.dma_start(
                    wf_sb[:],
                    d["wfold"][ridx * D : (ridx + 1) * D, :].rearrange(
                        "(dt p) c -> p dt c", p=128
                    ),
                )
                gl = [
                    (g, l)
                    for g in range(RATE_GROUPS[ridx])
                    for l in range(VP_COUNTS[g])
                ]
                for s2 in range(0, len(gl), 2):
                    ps = pp.tile([128, 1024], F32, tag="mm")
                    for half in range(2):
                        g, l = gl[s2 + half]
                        start = G_OFF[g] + G_STRIDE[g] * 128 * l
                        step = G_STRIDE[g]
                        end = min(start + 128 * step, S)
                        for dt in range(4):
                            nc.tensor.matmul(
                                ps[:, half * 512 : half * 512 + 512],
                                xtm_sb[:, dt, start:end:step],
                                wf_sb[:, dt, :],
                                start=(dt == 0),
                                stop=(dt == 3),
                            )
                    base = VP_BASE[ridx] + s2
                    dst = vp_sb[:, base : base + 2, :].rearrange("p a b -> p (a b)")
                    if s2 % 4 == 0:
                        nc.scalar.copy(dst, ps[:])
                    else:
                        nc.vector.tensor_copy(dst, ps[:])

        # ---------------- constants (scalar queue) ----------------
        def ld(name, shape):
            t = cpool.tile(shape, F32, tag=name)
            nc.scalar.dma_start(t[:], d[name])
            return t

        iidx_sb = ld("iidx", [128, ROWS])
        jidx_sb = ld("jidx", [128, N_SLOTS])
        slotsel_sb = cpool.tile([128, N_SLOTS, 4], F32R, tag="slotsel")
        nc.scalar.dma_start(slotsel_sb[:], d["slotsel"])
        bias_tile_sb = ld("bias_tile", [128, D])
        ident = cpool.tile([128, 128], F32R, tag="ident")
        nc.scalar.dma_start(ident[:], d["identr"])
        identf = cpool.tile([4, 4], F32, tag="identf")
        make_identity(nc, identf[:])

        with (
            tc.tile_pool(name="et", bufs=1) as et_pool,
            tc.tile_pool(name="persist", bufs=1) as persist,
            tc.tile_pool(name="avgp", bufs=2) as avg_pool,
        ):
            # ------- phase 3: transposed scores -> expT + group denominators -------
            # et slot s covers block1 columns 256:512 always; shared slots also
            # cover block0 in columns 0:256 (block0's slot list is a subset).
            et_sb = et_pool.tile([128, N_SLOTS, ROWS], F32R)
            sden = pp_acc.tile([4, ROWS], F32, tag="sden")
            for s, (g, l) in enumerate(SLOTS_B1):
                shared = (g, l) in SHARED
                c0 = 0 if shared else BW
                w = ROWS - c0
                start = G_OFF[g] + G_STRIDE[g] * 128 * l
                step = G_STRIDE[g]
                end = min(start + 128 * step, S)
                ps = pp.tile([128, ROWS], F32, tag="mm")
                for dt in range(4):
                    nc.tensor.matmul(
                        ps[:, 0:w],
                        gt_sb[:, dt, start:end:step],
                        xtq_sb[:, dt, c0:ROWS],
                        start=(dt == 0),
                        stop=(dt == 3),
                    )
                nc.scalar.activation(
                    et_sb[:, s, c0:ROWS], ps[:, 0:w], Exp, scale=1.0 / SQRT_D
                )
                nc.vector.scalar_tensor_tensor(
                    out=et_sb[:, s, c0:ROWS],
                    in0=iidx_sb[:, c0:ROWS],
                    scalar=jidx_sb[:, s : s + 1],
                    in1=et_sb[:, s, c0:ROWS],
                    op0=mybir.AluOpType.is_ge,
                    op1=mybir.AluOpType.mult,
                )
                nc.tensor.matmul(
                    sden[:, c0:ROWS],
                    slotsel_sb[:, s, :],
                    et_sb[:, s, c0:ROWS],
                    start=(s == 0),
                    stop=(s == N_SLOTS - 1),
                    skip_group_check=True,
                )

            # ------- phase 4: denominators -> reciprocals -------
            sden_sb = persist.tile([4, ROWS], F32, tag="sden_sb")
            nc.scalar.copy(sden_sb[:], sden[:])
            dT = pp.tile([128, 4 * N_IT], F32, tag="mm")
            for it in range(N_IT):
                nc.tensor.transpose(
                    dT[:, it * 4 : it * 4 + 4],
                    sden_sb[:, it * 128 : it * 128 + 128],
                    identf[:],
                )
            den_sb = persist.tile([128, 4 * N_IT], F32, tag="den")
            nc.vector.tensor_copy(den_sb[:], dT[:])
            # suffix sums over groups A,B,C,D -> denominators for rates 8,4,2,1
            for k in range(1, 4):
                nc.vector.tensor_add(
                    den_sb[:, k::4], den_sb[:, k::4], den_sb[:, k - 1 :: 4]
                )
            recip_sb = persist.tile([128, 4 * N_IT], F32, tag="recip")
            nc.vector.reciprocal(recip_sb[:], den_sb[:])
            # gcum[cls] = 0.25 * cumulative recips in rate order 1,2,4,8
            gcum_sb = persist.tile([128, 4 * N_IT], F32, tag="gcum")
            nc.vector.tensor_scalar_mul(gcum_sb[:, 0::4], recip_sb[:, 3::4], 0.25)
            rq = persist.tile([128, 4 * N_IT], F32, tag="rq")
            nc.vector.tensor_scalar_mul(rq[:], recip_sb[:], 0.25)
            for cls, k in ((1, 2), (2, 1), (3, 0)):
                nc.vector.tensor_add(
                    gcum_sb[:, cls::4], gcum_sb[:, cls - 1 :: 4], rq[:, k::4]
                )

            # ------- phases 5+6 interleaved per i-tile: Y/out then avg -------
            # avg[i, j in group g] = expT[j, i] * gcum[class(g)][i]; each group
            # is a residue class so the coefficient is a per-partition scalar.
            GRP_CLS = [3, 2, 1, 0]  # group A,B,C,D -> gcum class column
            out_sb = persist.tile([128, N_IT, D], F32, tag="out")
            for blk in (1, 0):
                slots = BLK_SLOTS[blk]
                n_jc = 2 if blk == 0 else 4
                for itl in range(2):
                    it = blk * 2 + itl
                    # Y_r and output rows for this i-tile
                    for ridx in range(4):
                        rs = [
                            (g, l) for (g, l) in slots if g < RATE_GROUPS[ridx]
                        ]
                        psy = pp.tile([128, D], F32, tag="mm")
                        for n, (g, l) in enumerate(rs):
                            col = blk * BW + itl * 128
                            nc.tensor.matmul(
                                psy[:],
                                et_sb[:, SLOTS_B1.index((g, l)), col : col + 128],
                                vp_sb[:, vp_index(ridx, g, l), :],
                                start=(n == 0),
                                stop=(n == len(rs) - 1),
                            )
                        col = it * 4 + (3 - ridx)
                        nc.vector.scalar_tensor_tensor(
                            out=out_sb[:, it, :],
                            in0=psy[:],
                            scalar=recip_sb[:, col : col + 1],
                            in1=(bias_tile_sb[:] if ridx == 0 else out_sb[:, it, :]),
                            op0=mybir.AluOpType.mult,
                            op1=mybir.AluOpType.add,
                        )
                    nc.sync.dma_start(
                        out_d[it * 128 : it * 128 + 128, :], out_sb[:, it, :]
                    )
                    # avg_attention rows for this i-tile
                    avg_sb = avg_pool.tile([128, 512 * n_jc], F32, tag=f"avg{blk}")
                    col = blk * BW + itl * 128
                    for s2 in range(0, len(slots), 8):
                        pst = pp.tile([128, 1024], F32R, tag="mm")
                        for k in range(8):
                            g, l = slots[s2 + k]
                            nc.tensor.transpose(
                                pst[:, k * 128 : k * 128 + 128],
                                et_sb[:, SLOTS_B1.index((g, l)), col : col + 128],
                                ident[:],
                            )
                        # adjacent same-group transposes form one strided dest
                        # run with a shared per-partition scalar -> batch them
                        runs = []
                        for k in range(8):
                            g, l = slots[s2 + k]
                            if runs and runs[-1][0] == g:
                                runs[-1][2] += 1
                            else:
                                runs.append([g, k, 1])
                        for g, k0, n in runs:
                            step = G_STRIDE[g]
                            end = min(G_OFF[g] + step * 128 * n, 512 * n_jc)
                            nc.vector.tensor_scalar_mul(
                                avg_sb[:, G_OFF[g] : end : step],
                                pst[:, k0 * 128 : (k0 + n) * 128],
                                gcum_sb[
                                    :, it * 4 + GRP_CLS[g] : it * 4 + GRP_CLS[g] + 1
                                ],
                            )
                    nc.sync.dma_start(
                        avg_d[it * 128 : it * 128 + 128, 0 : 512 * n_jc], avg_sb[:]
                    )


_CACHED_NC = None


def kernel(**inputs):
    global _CACHED_NC
    x = np.asarray(inputs["x"], np.float32)
    Wq, Wk, Wv = (np.asarray(inputs[k], np.float32) for k in ("Wq", "Wk", "Wv"))
    Wr, Wo = np.asarray(inputs["Wr"], np.float32), np.asarray(inputs["Wo"], np.float32)
    bq, bk, bv = (np.asarray(inputs[k], np.float32) for k in ("bq", "bk", "bv"))
    br, bo = np.asarray(inputs["br"], np.float32), np.asarray(inputs["bo"], np.float32)
    assert np.abs(bq).max() == 0 and np.abs(bk).max() == 0, (
        "kernel assumes bq=bk=0 (score cross-terms not implemented)"
    )

    W2 = np.ascontiguousarray((Wk @ Wq.T).astype(np.float32))
    Wfold = np.concatenate(
        [Wv @ Wr[r] @ Wo[r * D : (r + 1) * D, :] for r in range(4)]
    ).astype(np.float32)
    bias_out = (
        bo + sum((bv @ Wr[r] + br[r]) @ Wo[r * D : (r + 1) * D, :] for r in range(4))
    ).astype(np.float32)

    p = np.arange(128)
    jidx = np.zeros((128, N_SLOTS), np.float32)
    slotsel = np.zeros((128, N_SLOTS, 4), np.float32)
    for s, (g, l) in enumerate(SLOTS_B1):
        jidx[:, s] = slot_j(g, l, p)
        slotsel[:, s, g] = 1.0
    bias_tile = np.tile(bias_out, (128, 1)).astype(np.float32)
    identr = np.eye(128, dtype=np.float32)

    in_maps = []
    blocks = []
    for c in range(N_CORES):
        b, mp = c // 4, c % 4
        i0b = (BW * mp, BW * (7 - mp))
        blocks.append((b, i0b))
        xt = np.ascontiguousarray(x[b].T)
        xtq = np.concatenate(
            [xt[:, i0b[0] : i0b[0] + BW], xt[:, i0b[1] : i0b[1] + BW]], axis=1
        )
        iidx = np.tile(
            np.concatenate(
                [i0b[0] + np.arange(BW), i0b[1] + np.arange(BW)]
            ).astype(np.float32),
            (128, 1),
        )
        in_maps.append(
            {
                "xtm": xt,
                "xtq": np.ascontiguousarray(xtq),
                "w2": W2,
                "wfold": Wfold,
                "iidx": iidx,
                "jidx": jidx,
                "slotsel": slotsel,
                "bias_tile": bias_tile,
                "identr": identr,
            }
        )

    if _CACHED_NC is None:
        _CACHED_NC = build_program()
    res = run_bass_kernel_spmd(_CACHED_NC, in_maps, core_ids=list(range(N_CORES)))

    output = np.zeros((B, S, D), np.float32)
    avg = np.zeros((B, S, S), np.float32)
    for c in range(N_CORES):
        b, i0b = blocks[c]
        for blk in range(2):
            r0 = i0b[blk]
            output[b, r0 : r0 + BW, :] = res.results[c]["out"][
                blk * BW : blk * BW + BW
            ]
            avg[b, r0 : r0 + BW, :] = res.results[c]["avg"][blk * BW : blk * BW + BW]
    return output, avg


if __name__ == "__main__":
    import reference

    inputs = {k: np.asarray(v) for k, v in reference.setup_inputs().items()}
    out, avg = kernel(**inputs)
    print("kernel ran:", out.shape, avg.shape)


# revision 33
# speedup vs baseline: 1.0466x; 1.0466x over previous
"""DilatedAttention Trainium2 Bass kernel.

Reference math (B=2, S=2048, D=512, rates (1,2,4,8)):
  Q=xWq+bq K=xWk+bk V=xWv+bv
  scores = QK^T/sqrt(D); per rate r: mask j%r==0 & j<=i, softmax, ctx=attn@V,
  out_r = ctx@Wr[r]+br[r]; output = concat(out_r)@Wo + bo
  avg_attention = mean_r attn_r

Kernel strategy (8 NeuronCores, one uniform SPMD program):
  - core c handles batch c//4 with TWO 256-row query blocks m'=c%4 and 7-m'
    (the pairing balances causal work across cores; block0 rows < 1024).
  - host folds weights:  W2 = Wk@Wq^T  (scoresT[j,i] = x_j W2 x_i^T)
                         Wfold_r = Wv@Wr[r]@Wo_r
                         bias_out = bo + sum_r (bv@Wr[r]+br[r])@Wo_r
    (softmax rows summing to 1 turns the bv term into a constant; bq=bk=0
    asserted - they are structurally zero in this model.)
  - device: GT = W2^T@xT; transposed-layout scores per dilation *group*
    (A: j%8==0, B: j%8==4, C: j%4==2, D: j%2==1 -> strided SBUF column reads;
    a group is a residue class, so per-rate row sets are unions of leading
    groups), causal mask via scalar_tensor_tensor index compare, group
    denominators via one small matmul per slot, avg_attention produced by
    TensorE-transposing the masked exp tiles and scaling with the
    per-partition reciprocal-denominator combination of the slot's class,
    Y_r = expT_r @ (x_r@Wfold_r) with 1/denom fused into the output
    accumulation (scalar_tensor_tensor).
  - all matmuls float32r.
"""
import sys

sys.path.insert(0, "/opt/trn_rl_repo")

import math
import numpy as np

import concourse.bacc as bacc
import concourse.mybir as mybir
import concourse.tile as tile
from concourse.bass_utils import run_bass_kernel_spmd
from concourse.masks import make_identity

B, S, D = 2, 2048, 512
N_CORES = 8
ROWS = 512  # query rows per core (2 blocks of 256)
BW = 256  # block width
SQRT_D = math.sqrt(D)
F32 = mybir.dt.float32
F32R = mybir.dt.float32r

# dilation groups: j = stride*(128*local + p) + off
G_STRIDE = [8, 8, 4, 2]
G_OFF = [0, 4, 2, 1]
# block0 slot list sized for J<=1024; block1 full; block0 subset of block1
SLOTS_B0 = [(0, 0), (1, 0), (2, 0), (2, 1), (3, 0), (3, 1), (3, 2), (3, 3)]
SLOTS_B1 = (
    [(0, l) for l in range(2)] + [(1, l) for l in range(2)]
    + [(2, l) for l in range(4)] + [(3, l) for l in range(8)]
)
BLK_SLOTS = [SLOTS_B0, SLOTS_B1]
SHARED = set(SLOTS_B0)
N_SLOTS = len(SLOTS_B1)  # 16 physical slots; shared ones hold both blocks
VP_COUNTS = [2, 2, 4, 8]
VP_BASE = [0, 16, 24, 28]
N_VP = 30
N_IT = 4  # i-tiles of 128 (2 per block)
RATE_GROUPS = [4, 3, 2, 1]  # ridx (rates 1,2,4,8) -> number of leading groups


def vp_index(ridx, g, l):
    return VP_BASE[ridx] + sum(VP_COUNTS[:g]) + l


def slot_j(g, l, p):
    return G_STRIDE[g] * (128 * l + p) + G_OFF[g]


def build_program():
    nc = bacc.Bacc(
        "TRN2", target_bir_lowering=False, debug=False, num_devices=N_CORES
    )
    d = {}

    def inp(name, shape, dt=F32):
        d[name] = nc.dram_tensor(name, shape, dt, kind="ExternalInput").ap()

    inp("xtm", [D, S], F32R)
    inp("xtq", [D, ROWS], F32R)
    inp("w2", [D, D], F32R)
    inp("wfold", [4 * D, D], F32R)
    inp("iidx", [128, ROWS])
    inp("jidx", [128, N_SLOTS])
    inp("slotsel", [128, N_SLOTS, 4], F32R)
    inp("bias_tile", [128, D])
    inp("identr", [128, 128], F32R)
    out_d = nc.dram_tensor("out", [ROWS, D], F32, kind="ExternalOutput").ap()
    avg_d = nc.dram_tensor("avg", [ROWS, S], F32, kind="ExternalOutput").ap()

    with tile.TileContext(nc) as tc:
        build_core(tc, d, out_d, avg_d)
    nc.compile()
    return nc


def build_core(tc, d, out_d, avg_d):
    nc = tc.nc
    Exp = mybir.ActivationFunctionType.Exp

    with (
        tc.tile_pool(name="consts", bufs=1) as cpool,
        tc.tile_pool(name="gt", bufs=1) as gt_pool,
        tc.tile_pool(name="vp", bufs=1) as vp_pool,
        tc.tile_pool(name="psmm", bufs=3, space="PSUM") as pp,
        tc.tile_pool(name="psacc", bufs=1, space="PSUM") as pp_acc,
    ):
        gt_sb = gt_pool.tile([128, 4, S], F32R)
        vp_sb = vp_pool.tile([128, N_VP, D], F32R)

        # ------------- phases 1+2: GT and V' (xtm/weights scoped) -------------
        with (
            tc.tile_pool(name="xtm", bufs=1) as xtm_pool,
            tc.tile_pool(name="wstream", bufs=2) as wstream,
        ):
            w2_sb = wstream.tile([128, 4, D], F32R, tag="w")
            w2_dram = d["w2"].rearrange("(dt p) c -> p dt c", p=128)
            xtm_sb = xtm_pool.tile([128, 4, S], F32R)
            xtm_dram = d["xtm"].rearrange("(dt p) c -> p dt c", p=128)
            xtm0_sb = wstream.tile([128, 4, 512], F32R, tag="x0")
            for dt in range(4):
                nc.sync.dma_start(w2_sb[:, dt, :], w2_dram[:, dt, :])
                nc.sync.dma_start(xtm0_sb[:, dt, :], xtm_dram[:, dt, 0:512])
            xtq_sb = cpool.tile([128, 4, ROWS], F32R, tag="xtq")
            nc.scalar.dma_start(
                xtq_sb[:], d["xtq"].rearrange("(dt p) c -> p dt c", p=128)
            )
            for jc4 in range(4):
                nc.sync.dma_start(
                    xtm_sb[:, :, jc4 * 512 : jc4 * 512 + 512],
                    xtm_dram[:, :, jc4 * 512 : jc4 * 512 + 512],
                )

            # GT[dout, j] = W2^T @ xT  (dt outermost so the first matmuls only
            # need the dt=0 slices; chunk 0 reads its own small tile)
            for jc in range(4):
                col = jc * 512
                psa = pp.tile([128, 1024], F32, tag="mm")
                psb = pp.tile([128, 1024], F32, tag="mm")
                for dt in range(4):
                    for dto in range(4):
                        ps = psa if dto < 2 else psb
                        h = (dto % 2) * 512
                        nc.tensor.matmul(
                            ps[:, h : h + 512],
                            w2_sb[:, dt, dto * 128 : dto * 128 + 128],
                            (
                                xtm0_sb[:, dt, :]
                                if jc == 0
                                else xtm_sb[:, dt, col : col + 512]
                            ),
                            start=(dt == 0),
                            stop=(dt == 3),
                        )
                for dto in range(4):
                    ps = psa if dto < 2 else psb
                    h = (dto % 2) * 512
                    dst = gt_sb[:, dto, col : col + 512]
                    if dto % 2 == 0:
                        nc.scalar.copy(dst, ps[:, h : h + 512])
                    else:
                        nc.vector.tensor_copy(dst, ps[:, h : h + 512])

            # V'[vp slot] = x_rows(group tile) @ Wfold_r
            for ridx in range(4):
                wf_sb = wstream.tile([128, 4, D], F32R, tag="w")
                nc.scalar.dma_start(
                    wf_sb[:],
                    d["wfold"][ridx * D : (ridx + 1) * D, :].rearrange(
                        "(dt p) c -> p dt c", p=128
                    ),
                )
                gl = [
                    (g, l)
                    for g in range(RATE_GROUPS[ridx])
                    for l in range(VP_COUNTS[g])
                ]
                for s2 in range(0, len(gl), 2):
                    ps = pp.tile([128, 1024], F32, tag="mm")
                    for half in range(2):
                        g, l = gl[s2 + half]
                        start = G_OFF[g] + G_STRIDE[g] * 128 * l
                        step = G_STRIDE[g]
                        end = min(start + 128 * step, S)
                        for dt in range(4):
                            nc.tensor.matmul(
                                ps[:, half * 512 : half * 512 + 512],
                                xtm_sb[:, dt, start:end:step],
                                wf_sb[:, dt, :],
                                start=(dt == 0),
                                stop=(dt == 3),
                            )
                    base = VP_BASE[ridx] + s2
                    dst = vp_sb[:, base : base + 2, :].rearrange("p a b -> p (a b)")
                    if s2 % 4 == 0:
                        nc.scalar.copy(dst, ps[:])
                    else:
                        nc.vector.tensor_copy(dst, ps[:])

        # ---------------- constants (scalar queue) ----------------
        def ld(name, shape):
            t = cpool.tile(shape, F32, tag=name)
            nc.scalar.dma_start(t[:], d[name])
            return t

        iidx_sb = ld("iidx", [128, ROWS])
        jidx_sb = ld("jidx", [128, N_SLOTS])
        slotsel_sb = cpool.tile([128, N_SLOTS, 4], F32R, tag="slotsel")
        nc.scalar.dma_start(slotsel_sb[:], d["slotsel"])
        bias_tile_sb = ld("bias_tile", [128, D])
        ident = cpool.tile([128, 128], F32R, tag="ident")
        nc.scalar.dma_start(ident[:], d["identr"])
        identf = cpool.tile([4, 4], F32, tag="identf")
        make_identity(nc, identf[:])

        with (
            tc.tile_pool(name="et", bufs=1) as et_pool,
            tc.tile_pool(name="persist", bufs=1) as persist,
            tc.tile_pool(name="avgp", bufs=2) as avg_pool,
        ):
            # ------- phase 3: transposed scores -> expT + group denominators -------
            # et slot s covers block1 columns 256:512 always; shared slots also
            # cover block0 in columns 0:256 (block0's slot list is a subset).
            et_sb = et_pool.tile([128, N_SLOTS, ROWS], F32R)
            sden = pp_acc.tile([4, ROWS], F32, tag="sden")
            for s, (g, l) in enumerate(SLOTS_B1):
                shared = (g, l) in SHARED
                c0 = 0 if shared else BW
                w = ROWS - c0
                start = G_OFF[g] + G_STRIDE[g] * 128 * l
                step = G_STRIDE[g]
                end = min(start + 128 * step, S)
                ps = pp.tile([128, ROWS], F32, tag="mm")
                for dt in range(4):
                    nc.tensor.matmul(
                        ps[:, 0:w],
                        gt_sb[:, dt, start:end:step],
                        xtq_sb[:, dt, c0:ROWS],
                        start=(dt == 0),
                        stop=(dt == 3),
                    )
                nc.scalar.activation(
                    et_sb[:, s, c0:ROWS], ps[:, 0:w], Exp, scale=1.0 / SQRT_D
                )
                nc.vector.scalar_tensor_tensor(
                    out=et_sb[:, s, c0:ROWS],
                    in0=iidx_sb[:, c0:ROWS],
                    scalar=jidx_sb[:, s : s + 1],
                    in1=et_sb[:, s, c0:ROWS],
                    op0=mybir.AluOpType.is_ge,
                    op1=mybir.AluOpType.mult,
                )
                nc.tensor.matmul(
                    sden[:, c0:ROWS],
                    slotsel_sb[:, s, :],
                    et_sb[:, s, c0:ROWS],
                    start=(s == 0),
                    stop=(s == N_SLOTS - 1),
                    skip_group_check=True,
                )

            # ------- phase 4: denominators -> reciprocals -------
            sden_sb = persist.tile([4, ROWS], F32, tag="sden_sb")
            nc.scalar.copy(sden_sb[:], sden[:])
            dT = pp.tile([128, 4 * N_IT], F32, tag="mm")
            for it in range(N_IT):
                nc.tensor.transpose(
                    dT[:, it * 4 : it * 4 + 4],
                    sden_sb[:, it * 128 : it * 128 + 128],
                    identf[:],
                )
            den_sb = persist.tile([128, 4 * N_IT], F32, tag="den")
            nc.vector.tensor_copy(den_sb[:], dT[:])
            # suffix sums over groups A,B,C,D -> denominators for rates 8,4,2,1
            for k in range(1, 4):
                nc.vector.tensor_add(
                    den_sb[:, k::4], den_sb[:, k::4], den_sb[:, k - 1 :: 4]
                )
            recip_sb = persist.tile([128, 4 * N_IT], F32, tag="recip")
            nc.vector.reciprocal(recip_sb[:], den_sb[:])
            # gcum[cls] = 0.25 * cumulative recips in rate order 1,2,4,8
            gcum_sb = persist.tile([128, 4 * N_IT], F32, tag="gcum")
            nc.vector.tensor_scalar_mul(gcum_sb[:, 0::4], recip_sb[:, 3::4], 0.25)
            rq = persist.tile([128, 4 * N_IT], F32, tag="rq")
            nc.vector.tensor_scalar_mul(rq[:], recip_sb[:], 0.25)
            for cls, k in ((1, 2), (2, 1), (3, 0)):
                nc.vector.tensor_add(
                    gcum_sb[:, cls::4], gcum_sb[:, cls - 1 :: 4], rq[:, k::4]
                )

            # ------- phases 5+6 interleaved per i-tile: Y/out then avg -------
            # avg[i, j in group g] = expT[j, i] * gcum[class(g)][i]; each group
            # is a residue class so the coefficient is a per-partition scalar.
            GRP_CLS = [3, 2, 1, 0]  # group A,B,C,D -> gcum class column
            out_sb = persist.tile([128, N_IT, D], F32, tag="out")
            for blk in (1, 0):
                slots = BLK_SLOTS[blk]
                n_jc = 2 if blk == 0 else 4
                for itl in range(2):
                    it = blk * 2 + itl
                    # Y_r and output rows for this i-tile
                    for ridx in range(4):
                        rs = [
                            (g, l) for (g, l) in slots if g < RATE_GROUPS[ridx]
                        ]
                        psy = pp.tile([128, D], F32, tag="mm")
                        for n, (g, l) in enumerate(rs):
                            col = blk * BW + itl * 128
                            nc.tensor.matmul(
                                psy[:],
                                et_sb[:, SLOTS_B1.index((g, l)), col : col + 128],
                                vp_sb[:, vp_index(ridx, g, l), :],
                                start=(n == 0),
                                stop=(n == len(rs) - 1),
                            )
                        col = it * 4 + (3 - ridx)
                        nc.vector.scalar_tensor_tensor(
                            out=out_sb[:, it, :],
                            in0=psy[:],
                            scalar=recip_sb[:, col : col + 1],
                            in1=(bias_tile_sb[:] if ridx == 0 else out_sb[:, it, :]),
                            op0=mybir.AluOpType.mult,
                            op1=mybir.AluOpType.add,
                        )
                    nc.sync.dma_start(
                        out_d[it * 128 : it * 128 + 128, :], out_sb[:, it, :]
                    )
                    # avg_attention rows for this i-tile
                    avg_sb = avg_pool.tile([128, 512 * n_jc], F32, tag=f"avg{blk}")
                    col = blk * BW + itl * 128
                    for s2 in range(0, len(slots), 8):
                        pst = pp.tile([128, 1024], F32R, tag="mm")
                        for k in range(8):
                            g, l = slots[s2 + k]
                            nc.tensor.transpose(
                                pst[:, k * 128 : k * 128 + 128],
                                et_sb[:, SLOTS_B1.index((g, l)), col : col + 128],
                                ident[:],
                            )
                        # adjacent same-group transposes form one strided dest
                        # run with a shared per-partition scalar -> batch them
                        runs = []
                        for k in range(8):
                            g, l = slots[s2 + k]
                            if runs and runs[-1][0] == g:
                                runs[-1][2] += 1
                            else:
                                runs.append([g, k, 1])
                        for g, k0, n in runs:
                            step = G_STRIDE[g]
                            end = min(G_OFF[g] + step * 128 * n, 512 * n_jc)
                            nc.vector.tensor_scalar_mul(
                                avg_sb[:, G_OFF[g] : end : step],
                                pst[:, k0 * 128 : (k0 + n) * 128],
                                gcum_sb[
                                    :, it * 4 + GRP_CLS[g] : it * 4 + GRP_CLS[g] + 1
                                ],
                            )
                    nc.sync.dma_start(
                        avg_d[it * 128 : it * 128 + 128, 0 : 512 * n_jc], avg_sb[:]
                    )


_CACHED_NC = None


def kernel(**inputs):
    global _CACHED_NC
    x = np.asarray(inputs["x"], np.float32)
    Wq, Wk, Wv = (np.asarray(inputs[k], np.float32) for k in ("Wq", "Wk", "Wv"))
    Wr, Wo = np.asarray(inputs["Wr"], np.float32), np.asarray(inputs["Wo"], np.float32)
    bq, bk, bv = (np.asarray(inputs[k], np.float32) for k in ("bq", "bk", "bv"))
    br, bo = np.asarray(inputs["br"], np.float32), np.asarray(inputs["bo"], np.float32)
    assert np.abs(bq).max() == 0 and np.abs(bk).max() == 0, (
        "kernel assumes bq=bk=0 (score cross-terms not implemented)"
    )

    W2 = np.ascontiguousarray((Wk @ Wq.T).astype(np.float32))
    Wfold = np.concatenate(
        [Wv @ Wr[r] @ Wo[r * D : (r + 1) * D, :] for r in range(4)]
    ).astype(np.float32)
    bias_out = (
        bo + sum((bv @ Wr[r] + br[r]) @ Wo[r * D : (r + 1) * D, :] for r in range(4))
    ).astype(np.float32)

    p = np.arange(128)
    jidx = np.zeros((128, N_SLOTS), np.float32)
    slotsel = np.zeros((128, N_SLOTS, 4), np.float32)
    for s, (g, l) in enumerate(SLOTS_B1):
        jidx[:, s] = slot_j(g, l, p)
        slotsel[:, s, g] = 1.0
    bias_tile = np.tile(bias_out, (128, 1)).astype(np.float32)
    identr = np.eye(128, dtype=np.float32)

    in_maps = []
    blocks = []
    for c in range(N_CORES):
        b, mp = c // 4, c % 4
        i0b = (BW * mp, BW * (7 - mp))
        blocks.append((b, i0b))
        xt = np.ascontiguousarray(x[b].T)
        xtq = np.concatenate(
            [xt[:, i0b[0] : i0b[0] + BW], xt[:, i0b[1] : i0b[1] + BW]], axis=1
        )
        iidx = np.tile(
            np.concatenate(
                [i0b[0] + np.arange(BW), i0b[1] + np.arange(BW)]
            ).astype(np.float32),
            (128, 1),
        )
        in_maps.append(
            {
                "xtm": xt,
                "xtq": np.ascontiguousarray(xtq),
                "w2": W2,
                "wfold": Wfold,
                "iidx": iidx,
                "jidx": jidx,
                "slotsel": slotsel,
                "bias_tile": bias_tile,
                "identr": identr,
            }
        )

    if _CACHED_NC is None:
        _CACHED_NC = build_program()
    res = run_bass_kernel_spmd(_CACHED_NC, in_maps, core_ids=list(range(N_CORES)))

    output = np.zeros((B, S, D), np.float32)
    avg = np.zeros((B, S, S), np.float32)
    for c in range(N_CORES):
        b, i0b = blocks[c]
        for blk in range(2):
            r0 = i0b[blk]
            output[b, r0 : r0 + BW, :] = res.results[c]["out"][
                blk * BW : blk * BW + BW
            ]
            avg[b, r0 : r0 + BW, :] = res.results[c]["avg"][blk * BW : blk * BW + BW]
    return output, avg


if __name__ == "__main__":
    import reference

    inputs = {k: np.asarray(v) for k, v in reference.setup_inputs().items()}
    out, avg = kernel(**inputs)
    print("kernel ran:", out.shape, avg.shape)
